# revision 5
# baseline (speedup 1.0000x reference)
"""DetConB loss (nn_DetConBLoss) on 8 TRN2 NeuronCores via Bass/Tile.

Strategy (data-parallel over batch, targets replicated):
  - Host: l2-normalize preds/targets in f32, flatten to (4096, 256),
    transpose to (d, rows), cast fp8. Core c owns pred rows
    [c*512, (c+1)*512). Each core receives the full targets with columns
    rolled by c*512 so its own-image diagonal band sits at a fixed,
    compile-time-constant column range (the program is SPMD-identical).
    Device tensors are laid out block-major ([p, block, k, col]) so each
    DMA lands as large per-partition-contiguous packets.
  - Device (per core): for each of the 4 pred x target combinations,
    a (512 x 4096) fp8 DoubleRow matmul (K=256 in one pass, fp32 PSUM
    accum). Per 128-row iteration the 4096 logit columns split:
      * cols [0, ACOLS): ScalarE exp (fused scale) with the ACT
        accumulator producing the row-sum for free.
      * cols [ACOLS, 4096): DVE Schraudolph fast-exp (int-converting
        multiply-add -> int32 bit pattern) followed by a tensor_scalar
        copy-with-accumulator over the f32 bitcast (all-SBUF operands ->
        DVE 2x mode) producing that row-sum.
    Only the 16 KB of row-sum partials leave the device.
  - Host: the 16x16 own-image diagonal dot blocks (recomputed from the
    same fp8 inputs, ~0.4% of total FLOPs), masks from the roi indices,
    positive-pair sums, the -inf masking correction (subtract the exp of
    masked entries from the denominators), log, and the final mean.
"""
import numpy as np
import ml_dtypes

import concourse.bacc as bacc
import concourse.mybir as mybir
import concourse.tile as tile
from concourse.bass_utils import run_bass_kernel_spmd

TEMP = 0.1
EPS = 1e-11
SCALE = float(np.float32(1.0 / (TEMP + EPS)))
NCORES = 8
B, N, D = 256, 16, 256
R = B * N          # 4096 flat rows
RPC = R // NCORES  # 512 rows per core
MT = RPC // 128    # 4 row-tiles of 128 per core
BF16 = mybir.dt.bfloat16
FP8 = mybir.dt.float8e4
NPFP8 = ml_dtypes.float8_e4m3
F32 = mybir.dt.float32
I32 = mybir.dt.int32
# Column split per 128-row iteration: ScalarE takes ACOLS, DVE the rest.
# Chosen to equalize ScalarE (0.96 ns/col exp + 283 ns accumulator read)
# against DVE (1.14 ns/col Schraudolph + 1.19 ns/col reduce).
ACOLS = 2832
VCOLS = 4096 - ACOLS
# Schraudolph fast-exp: exp(s*x) ~= bitcast_f32(int32(x*SA + SB))
SA = float(np.float32((2**23 / np.log(2.0)) * (1.0 / (0.1 + 1e-11))))
SB = float(np.float32(127 * 2**23 - 486411))


def build_nc():
    """Build + schedule + compile the SPMD per-core Bass program."""
    nc = bacc.Bacc("TRN2", target_bir_lowering=False, debug=False,
                   num_devices=NCORES)

    # Block-major layouts (see host_prep): p [128, mt*k*128], t [128, j*k*512]
    p_dram = [nc.dram_tensor(f"p{i + 1}t", [128, 2 * RPC], FP8,
                             kind="ExternalInput") for i in range(2)]
    t_dram = [nc.dram_tensor(f"t{i + 1}t", [128, 2 * R], FP8,
                             kind="ExternalInput") for i in range(2)]
    sacc = nc.dram_tensor("sacc", [128, 40], F32, kind="ExternalOutput")

    with tile.TileContext(nc) as tc:
        with (
            tc.tile_pool(name="const", bufs=1) as const_pool,
            tc.tile_pool(name="psum", bufs=1, space="PSUM") as psum_pool,
            tc.tile_pool(name="scratch", bufs=2) as scratch_pool,
        ):
            t_sb = [const_pool.tile([128, 2 * R], FP8, name=f"t_sb{i}", tag=f"t{i}")
                    for i in range(2)]
            p_sb = [const_pool.tile([128, 2 * RPC], FP8, name=f"p_sb{i}", tag=f"p{i}")
                    for i in range(2)]

            # Row-sum partials: col 2*it = ACT accumulator, 2*it+1 = DVE.
            strip = const_pool.tile([128, 40], F32, name="strip", tag="strip")
            nc.vector.memset(strip, 0.0)
            # Explicit zero-bias AP: a float bias would be lowered through the
            # const-AP machinery, whose TENSOR_LOAD sits in the preamble.
            zbias = const_pool.tile([128, 1], F32, name="zbias", tag="zbias")
            nc.vector.memset(zbias, 0.0)
            # Warm the exp table set during the input-DMA window so the first
            # real ACTIVATE does not pay the ~2.7us ACT_TABLE_LOAD.
            nc.scalar.activation(strip[:, 0:2], strip[:, 0:2],
                                 mybir.ActivationFunctionType.Exp, bias=zbias)
            nc.vector.memset(strip[:, 0:2], 0.0)

            # Input DMAs. p loads ride the Activation HWDGE queue, t loads
            # the sync (SP) queue: descriptor generation (~0.6us per DMA)
            # runs in parallel on the two engines instead of serializing.
            def load_p(px):
                nc.scalar.dma_start(out=p_sb[px], in_=p_dram[px].ap())

            def load_t(tsel, j0, nj):
                cs = j0 * 1024
                nc.sync.dma_start(
                    out=t_sb[tsel][:, cs:cs + nj * 1024],
                    in_=t_dram[tsel][:, cs:cs + nj * 1024])

            load_p(0)
            load_p(1)
            # t1 in j-granular chunks so iteration 0's matmuls chase the DMA;
            # t2 afterwards (first needed ~25us in).
            for (j0, nj) in ((0, 1), (1, 1), (2, 2), (4, 2), (6, 2)):
                load_t(0, j0, nj)
            load_t(1, 0, 4)
            load_t(1, 4, 4)

            rhs4 = [t_sb[i].rearrange("p (j k c) -> p j k c", j=8, k=2)
                    for i in range(2)]
            lhs4 = [p_sb[i].rearrange("p (mt k c) -> p mt k c", mt=MT, k=2)
                    for i in range(2)]

            # tsel outer: the first 8 iterations consume only t1, so the t2
            # load (1 MB) hides behind ~22 us of compute.
            for tsel in range(2):
                for px in range(2):
                    for mt in range(MT):
                        it = tsel * 8 + px * MT + mt
                        # One PSUM tile spans all 8 banks; the ACOLS boundary
                        # splits it between the two consumer engines via
                        # subtile dependencies.
                        ps = psum_pool.tile([128, 4096], F32, name="ps",
                                            tag="ps")
                        # fp8 DoubleRow: both 128-deep K chunks contract in a
                        # single pass (lhsT/rhs carry the k pair on a middle
                        # AP dim), so each 512-col tile is one matmul.
                        for j in range(8):
                            nc.tensor.matmul(
                                ps[:, j * 512:(j + 1) * 512],
                                lhs4[px][:, mt],
                                rhs4[tsel][:, j],
                                start=True, stop=True,
                                perf_mode=mybir.MatmulPerfMode.DoubleRow)
                        # (The own-image diagonal blocks always fall in the
                        # ACT range; the host recomputes them from the same
                        # fp8 inputs — no band output needed.)
                        # fp8 exp output: the values only feed the in-ACT
                        # accumulator (tapped pre-conversion), so the cheapest
                        # legal output dtype minimizes SBUF write bandwidth,
                        # keeping the PE's fp8 rhs fetch at 2 elem/cycle.
                        scr0 = scratch_pool.tile([128, ACOLS], FP8,
                                                 name="scr0", tag="scr")
                        nc.scalar.activation(
                            scr0, ps[:, 0:ACOLS],
                            mybir.ActivationFunctionType.Exp,
                            bias=zbias, scale=SCALE,
                            accum_out=strip[:, 2 * it:2 * it + 1])
                        # DVE: Schraudolph int-construct at 1x (PSUM source),
                        # then a 1x reduce over the f32 bitcast.
                        sch = scratch_pool.tile([128, VCOLS], I32,
                                                name="sch", tag="sch")
                        nc.vector.tensor_scalar(
                            sch, ps[:, ACOLS:4096], SA, SB,
                            op0=mybir.AluOpType.mult,
                            op1=mybir.AluOpType.add)
                        nc.vector.tensor_reduce(
                            strip[:, 2 * it + 1:2 * it + 2], sch.bitcast(F32),
                            axis=mybir.AxisListType.X, op=mybir.AluOpType.add)
            # Final strip DMA on the sync HWDGE queue: the gpsimd SWDGE
            # drain at kernel exit is ~2.4us when it must wait for this
            # transfer; HWDGE drains in ~0.1us.
            nc.sync.dma_start(out=sacc.ap(), in_=strip)

    nc.compile()
    return nc


_NC = None


def _get_nc():
    global _NC
    if _NC is None:
        _NC = build_nc()
    return _NC


def _l2norm(x):
    return x / np.linalg.norm(x, axis=-1, keepdims=True)


def _dev_p_layout(pt):
    # pt: [D=256, RPC] fp8 -> [128, mt, k, 128] block-major
    return np.ascontiguousarray(
        pt.reshape(2, 128, MT, 128).transpose(1, 2, 0, 3).reshape(128, 2 * RPC))


def _dev_t_layout(tt):
    # tt: [D=256, R] fp8 -> [128, j, k, 512] block-major
    return np.ascontiguousarray(
        tt.reshape(2, 128, 8, 512).transpose(1, 2, 0, 3).reshape(128, 2 * R))


def host_prep(pred1, pred2, target1, target2):
    p1t = _l2norm(np.asarray(pred1, np.float32)).reshape(R, D).T.astype(NPFP8)
    p2t = _l2norm(np.asarray(pred2, np.float32)).reshape(R, D).T.astype(NPFP8)
    t1t = _l2norm(np.asarray(target1, np.float32)).reshape(R, D).T.astype(NPFP8)
    t2t = _l2norm(np.asarray(target2, np.float32)).reshape(R, D).T.astype(NPFP8)
    # Raw own-image diagonal dot blocks (b, n, m), fp8-quantized operands in
    # f32 — the same products the device computes, ~0.4% of total FLOPs.
    pf = [p1t.T.astype(np.float32).reshape(B, N, D),
          p2t.T.astype(np.float32).reshape(B, N, D)]
    tf = [t1t.T.astype(np.float32).reshape(B, N, D),
          t2t.T.astype(np.float32).reshape(B, N, D)]
    diag = [[np.einsum('bnd,bmd->bnm', pf[px], tf[ts]).astype(np.float32)
             for ts in range(2)] for px in range(2)]
    in_maps = []
    for c in range(NCORES):
        r0 = c * RPC
        in_maps.append({
            "p1t": _dev_p_layout(p1t[:, r0:r0 + RPC]),
            "p2t": _dev_p_layout(p2t[:, r0:r0 + RPC]),
            "t1t": _dev_t_layout(np.concatenate([t1t[:, r0:], t1t[:, :r0]], axis=1)),
            "t2t": _dev_t_layout(np.concatenate([t2t[:, r0:], t2t[:, :r0]], axis=1)),
        })
    return in_maps, diag


def host_post(results, diag, pind1, pind2, tind1, tind2):
    S = np.zeros((2, R), np.float64)
    for c, res in enumerate(results):
        sacc = np.asarray(res["sacc"])
        for px in range(2):
            for mt in range(MT):
                r0 = c * RPC + mt * 128
                cols = [2 * (tsel * 8 + px * MT + mt) + h
                        for tsel in range(2) for h in range(2)]
                S[px, r0:r0 + 128] = sacc[:, cols].astype(np.float64).sum(axis=1)
    sc = np.float32(SCALE)
    D_aa = sc * diag[0][0]
    D_ab = sc * diag[0][1]
    D_ba = sc * diag[1][0]
    D_bb = sc * diag[1][1]

    f32 = np.float32
    pind1, pind2 = np.asarray(pind1), np.asarray(pind2)
    tind1, tind2 = np.asarray(tind1), np.asarray(tind2)
    same_aa = (pind1[:, :, None] == tind1[:, None, :]).astype(f32)
    same_ab = (pind1[:, :, None] == tind2[:, None, :]).astype(f32)
    same_ba = (pind2[:, :, None] == tind1[:, None, :]).astype(f32)
    same_bb = (pind2[:, :, None] == tind2[:, None, :]).astype(f32)

    S0 = S[0].reshape(B, N)
    S1 = S[1].reshape(B, N)
    corr0 = (same_aa * np.exp(D_aa.astype(np.float64))).sum(-1)
    corr1 = (same_bb * np.exp(D_bb.astype(np.float64))).sum(-1)
    lse0 = np.log(S0 - corr0)
    lse1 = np.log(S1 - corr1)

    num_pos0 = same_ab.sum(-1)
    num_pos1 = same_ba.sum(-1)
    pos_sum0 = (same_ab * D_ab).sum(-1)
    pos_sum1 = (same_ba * D_ba).sum(-1)

    area0 = (pind1[:, :, None] == pind1[:, None, :]).astype(f32).sum(-1)
    area1 = (pind2[:, :, None] == pind2[:, None, :]).astype(f32).sum(-1)
    w0 = (num_pos0 > 0.001).astype(f32) / area0
    w1 = (num_pos1 > 0.001).astype(f32) / area1

    ce0 = -w0 * (pos_sum0 - num_pos0 * lse0) / np.maximum(num_pos0, 1.0)
    ce1 = -w1 * (pos_sum1 - num_pos1 * lse1) / np.maximum(num_pos1, 1.0)
    return np.float32(ce0.mean() + ce1.mean())


def run_hw(inputs, trace=False):
    nc = _get_nc()
    in_maps, diag = host_prep(inputs["pred1"], inputs["pred2"],
                              inputs["target1"], inputs["target2"])
    last_err = None
    for attempt in range(3):
        try:
            res = run_bass_kernel_spmd(nc, in_maps,
                                       core_ids=list(range(NCORES)),
                                       trace=trace)
            break
        except Exception as e:  # transient NRT device errors recover on retry
            last_err = e
            import time
            time.sleep(20 * (attempt + 1))
    else:
        raise last_err
    loss = host_post(res.results, diag, inputs["pind1"], inputs["pind2"],
                     inputs["tind1"], inputs["tind2"])
    return loss, res


def kernel(**inputs):
    loss, _ = run_hw(inputs, trace=False)
    return loss


# revision 7
# speedup vs baseline: 1.6832x; 1.6832x over previous
"""DetConB loss (nn_DetConBLoss) on 8 TRN2 NeuronCores via Bass/Tile.

Strategy (data-parallel over batch, targets replicated):
  - Host: l2-normalize preds/targets in f32, flatten to (4096, 256),
    transpose to (d, rows), cast bf16. Core c owns pred rows
    [c*512, (c+1)*512). Each core receives the full targets with columns
    rolled by c*512 so its own-image diagonal band sits at a fixed,
    compile-time-constant column range (the program is SPMD-identical).
  - Device (per core): for each of the 4 pred x target combinations,
    a (512 x 4096) fp8 DoubleRow matmul (K=256 in one pass, fp32 PSUM
    accum) fused with exp(scale*x) on ScalarE at its roofline; row-sums
    via ACTIVATE's accumulator on one PSUM buffer and a DVE reduce on
    the other. Only the 32 KB of row-sum partials leave the device.
  - Host: the 16x16 own-image diagonal dot blocks (recomputed from the
    same fp8 inputs, ~0.4% of total FLOPs), masks from the roi indices,
    positive-pair sums, the -inf masking correction (subtract the exp of
    masked entries from the denominators), log, and the final mean.

All 34.4 GFLOP of matmul and the 67M-element exp run on device; the host
handles O(b*n^2)-scale arithmetic.
"""
import numpy as np
import ml_dtypes

import concourse.bacc as bacc
import concourse.mybir as mybir
import concourse.tile as tile
from concourse.bass_utils import run_bass_kernel_spmd

TEMP = 0.1
EPS = 1e-11
SCALE = float(np.float32(1.0 / (TEMP + EPS)))
NCORES = 8
B, N, D = 256, 16, 256
R = B * N          # 4096 flat rows
RPC = R // NCORES  # 512 rows per core
MT = RPC // 128    # 4 row-tiles of 128 per core
BF16 = mybir.dt.bfloat16
FP8 = mybir.dt.float8e4
NPFP8 = ml_dtypes.float8_e4m3
F32 = mybir.dt.float32
I32 = mybir.dt.int32
# Schraudolph fast-exp: exp(s*x) ~= bitcast_f32(int32(x*SA + SB))
SA = float(np.float32((2**23 / np.log(2.0)) * (1.0 / (0.1 + 1e-11))))
SB = float(np.float32(127 * 2**23 - 486411))


def build_nc():
    """Build + schedule + compile the SPMD per-core Bass program."""
    nc = bacc.Bacc("TRN2", target_bir_lowering=False, debug=False,
                   num_devices=NCORES)

    p_dram = [nc.dram_tensor(f"p{i + 1}t", [D, RPC], FP8, kind="ExternalInput")
              for i in range(2)]
    t_dram = [nc.dram_tensor(f"t{i + 1}t", [D, R], FP8, kind="ExternalInput")
              for i in range(2)]
    sacc = nc.dram_tensor("sacc", [128, 80], F32, kind="ExternalOutput")

    with tile.TileContext(nc) as tc:
        with (
            tc.tile_pool(name="const", bufs=1) as const_pool,
            tc.tile_pool(name="psum", bufs=2, space="PSUM") as psum_pool,
            tc.tile_pool(name="scratch", bufs=6) as scratch_pool,
        ):
            # Persistent SBUF: targets as [K=128 partitions, kchunk*R + col],
            # preds as [128, kchunk*RPC + col].
            t_sb = [const_pool.tile([128, 2 * R], FP8, name=f"t_sb{i}", tag=f"t{i}")
                    for i in range(2)]
            p_sb = [const_pool.tile([128, 2 * RPC], FP8, name=f"p_sb{i}", tag=f"p{i}")
                    for i in range(2)]

            # All 32 row-sum partials live in one persistent strip; a single
            # 32 KB DMA ships them at the end (col 2*it = g0 via DVE reduce,
            # col 2*it+1 = g1 via ACT accumulator).
            strip = const_pool.tile([128, 80], F32, name="strip", tag="strip")
            nc.vector.memset(strip, 0.0)
            # Explicit zero-bias AP: a float bias would be lowered through the
            # const-AP machinery, whose TENSOR_LOAD sits in the preamble.
            zbias = const_pool.tile([128, 1], F32, name="zbias", tag="zbias")
            nc.vector.memset(zbias, 0.0)
            # Warm the exp table set during the input-DMA window so the first
            # real ACTIVATE does not pay the ~2.7us ACT_TABLE_LOAD.
            nc.scalar.activation(strip[:, 0:2], strip[:, 0:2],
                                 mybir.ActivationFunctionType.Exp, bias=zbias)
            nc.vector.memset(strip[:, 0:2], 0.0)

            # Input DMAs: t loads ride the sync (SP) HWDGE queue, p loads the
            # Activation queue — descriptor generation (~0.65us per DMA) runs
            # on both engines in parallel instead of serializing on SP.
            def load_t(tsel, k, g):
                cs = g * 2048
                nc.sync.dma_start(
                    out=t_sb[tsel][:, k * R + cs: k * R + cs + 2048],
                    in_=t_dram[tsel][k * 128:(k + 1) * 128, cs:cs + 2048])

            def load_p(px):
                nc.scalar.dma_start(
                    out=p_sb[px].rearrange("p (k c) -> p k c", k=2),
                    in_=p_dram[px].ap().rearrange("(k p) c -> p k c", p=128))

            def load_t_fine(tsel, k, g, q):
                cs = g * 2048 + q * 1024
                nc.sync.dma_start(
                    out=t_sb[tsel][:, k * R + cs: k * R + cs + 1024],
                    in_=t_dram[tsel][k * 128:(k + 1) * 128, cs:cs + 1024])

            load_p(0)
            load_p(1)
            for q in range(2):
                load_t_fine(0, 0, 0, q)
                load_t_fine(0, 1, 0, q)
            load_t(0, 0, 1)
            load_t(0, 1, 1)
            for k in range(2):
                for g in range(2):
                    load_t(1, k, g)

            # tsel outer: the first 8 halves consume only t1, so the t2
            # load (2 MB) hides behind ~28 us of compute.
            for tsel in range(2):
                for px in range(2):
                    for mt in range(MT):
                        it = tsel * 8 + px * MT + mt
                        # One 4096-col half = both PSUM buffers. k-outer so 8
                        # consecutive matmuls share the stationary weights and
                        # stream back-to-back (no LDWEIGHTS-induced drain).
                        ps = [psum_pool.tile([128, 2048], F32, name=f"ps{h}",
                                             tag="ps")
                              for h in range(2)]
                        # fp8 DoubleRow: both 128-deep K chunks contract in a
                        # single pass (lhsT/rhs carry the k pair on a middle
                        # AP dim), so each 512-col tile is one matmul.
                        lhs3 = p_sb[px].rearrange("p (k c) -> p k c", k=2)
                        rhs3 = t_sb[tsel].rearrange("p (k c) -> p k c", k=2)
                        for g in range(2):
                            for j in range(4):
                                c0 = g * 2048 + j * 512
                                nc.tensor.matmul(
                                    ps[g][:, j * 512:(j + 1) * 512],
                                    lhs3[:, :, mt * 128:(mt + 1) * 128],
                                    rhs3[:, :, c0:c0 + 512],
                                    start=True, stop=True,
                                    perf_mode=mybir.MatmulPerfMode.DoubleRow)
                        # (The own-image diagonal blocks are recomputed on
                        # the host from the same fp8 inputs — no band output.)
                        # g0: ACT accumulator (its READ_ACCUMULATOR lands
                        # mid-period, off the inter-iteration critical path);
                        # g1: exp on ACT, row-sum on the otherwise-idle DVE.
                        scr0 = scratch_pool.tile([128, 2048], BF16, name="scr0",
                                                 tag="scr")
                        nc.scalar.activation(
                            scr0, ps[0], mybir.ActivationFunctionType.Exp,
                            bias=zbias, scale=SCALE,
                            accum_out=strip[:, 2 * it:2 * it + 1])
                        scr1 = scratch_pool.tile([128, 2048], BF16, name="scr1",
                                                 tag="scr")
                        if it == 15:
                            # Final iteration: DVE work would sit on the
                            # kernel-exit path; the ACT accumulator's read-out
                            # is cheaper there.
                            nc.scalar.activation(
                                scr1, ps[1], mybir.ActivationFunctionType.Exp,
                                bias=zbias, scale=SCALE,
                                accum_out=strip[:, 2 * it + 1:2 * it + 2])
                        else:
                            # Half of g1 goes through a Schraudolph fast-exp
                            # on the now-idle DVE (int-converting multiply-add
                            # + reduce of the bitcast), shortening the
                            # critical ScalarE chain to 2048+1024 columns.
                            sch = scratch_pool.tile([128, 1024], I32,
                                                    name="sch", tag="sch")
                            nc.vector.tensor_scalar(
                                sch, ps[1][:, 1024:2048], SA, SB,
                                op0=mybir.AluOpType.mult,
                                op1=mybir.AluOpType.add)
                            nc.vector.tensor_reduce(
                                strip[:, 64 + it:65 + it], sch.bitcast(F32),
                                axis=mybir.AxisListType.X, op=mybir.AluOpType.add)
                            nc.scalar.activation(
                                scr1[:, 0:1024], ps[1][:, 0:1024],
                                mybir.ActivationFunctionType.Exp,
                                bias=zbias, scale=SCALE)
                            nc.vector.tensor_reduce(
                                strip[:, 2 * it + 1:2 * it + 2], scr1[:, 0:1024],
                                axis=mybir.AxisListType.X, op=mybir.AluOpType.add)
            # Final strip DMA on the sync HWDGE queue: the gpsimd SWDGE
            # drain at kernel exit is ~2.4us when it must wait for this
            # transfer; HWDGE drains in ~0.1us.
            nc.sync.dma_start(out=sacc.ap(), in_=strip)

    nc.compile()
    return nc


_NC = None


def _get_nc():
    global _NC
    if _NC is None:
        _NC = build_nc()
    return _NC


def _l2norm(x):
    return x / np.linalg.norm(x, axis=-1, keepdims=True)


def host_prep(pred1, pred2, target1, target2):
    p1t = _l2norm(np.asarray(pred1, np.float32)).reshape(R, D).T.astype(NPFP8)
    p2t = _l2norm(np.asarray(pred2, np.float32)).reshape(R, D).T.astype(NPFP8)
    t1t = _l2norm(np.asarray(target1, np.float32)).reshape(R, D).T.astype(NPFP8)
    t2t = _l2norm(np.asarray(target2, np.float32)).reshape(R, D).T.astype(NPFP8)
    # Raw own-image diagonal dot blocks (b, n, m), fp8-quantized operands in
    # f32 — the same products the device computes, ~0.4% of total FLOPs.
    pf = [p1t.T.astype(np.float32).reshape(B, N, D),
          p2t.T.astype(np.float32).reshape(B, N, D)]
    tf = [t1t.T.astype(np.float32).reshape(B, N, D),
          t2t.T.astype(np.float32).reshape(B, N, D)]
    diag = [[np.einsum('bnd,bmd->bnm', pf[px], tf[ts]).astype(np.float32)
             for ts in range(2)] for px in range(2)]
    in_maps = []
    for c in range(NCORES):
        r0 = c * RPC
        in_maps.append({
            "p1t": np.ascontiguousarray(p1t[:, r0:r0 + RPC]),
            "p2t": np.ascontiguousarray(p2t[:, r0:r0 + RPC]),
            "t1t": np.ascontiguousarray(np.concatenate([t1t[:, r0:], t1t[:, :r0]], axis=1)),
            "t2t": np.ascontiguousarray(np.concatenate([t2t[:, r0:], t2t[:, :r0]], axis=1)),
        })
    return in_maps, diag


def host_post(results, diag, pind1, pind2, tind1, tind2):
    S = np.zeros((2, R), np.float64)
    for c, res in enumerate(results):
        sacc = np.asarray(res["sacc"])
        for px in range(2):
            for mt in range(MT):
                r0 = c * RPC + mt * 128
                cols = [2 * (tsel * 8 + px * MT + mt) + g
                        for tsel in range(2) for g in range(2)]
                cols += [64 + tsel * 8 + px * MT + mt for tsel in range(2)]
                S[px, r0:r0 + 128] = sacc[:, cols].astype(np.float64).sum(axis=1)
    sc = np.float32(SCALE)
    D_aa = sc * diag[0][0]
    D_ab = sc * diag[0][1]
    D_ba = sc * diag[1][0]
    D_bb = sc * diag[1][1]

    f32 = np.float32
    pind1, pind2 = np.asarray(pind1), np.asarray(pind2)
    tind1, tind2 = np.asarray(tind1), np.asarray(tind2)
    same_aa = (pind1[:, :, None] == tind1[:, None, :]).astype(f32)
    same_ab = (pind1[:, :, None] == tind2[:, None, :]).astype(f32)
    same_ba = (pind2[:, :, None] == tind1[:, None, :]).astype(f32)
    same_bb = (pind2[:, :, None] == tind2[:, None, :]).astype(f32)

    S0 = S[0].reshape(B, N)
    S1 = S[1].reshape(B, N)
    corr0 = (same_aa * np.exp(D_aa.astype(np.float64))).sum(-1)
    corr1 = (same_bb * np.exp(D_bb.astype(np.float64))).sum(-1)
    lse0 = np.log(S0 - corr0)
    lse1 = np.log(S1 - corr1)

    num_pos0 = same_ab.sum(-1)
    num_pos1 = same_ba.sum(-1)
    pos_sum0 = (same_ab * D_ab).sum(-1)
    pos_sum1 = (same_ba * D_ba).sum(-1)

    area0 = (pind1[:, :, None] == pind1[:, None, :]).astype(f32).sum(-1)
    area1 = (pind2[:, :, None] == pind2[:, None, :]).astype(f32).sum(-1)
    w0 = (num_pos0 > 0.001).astype(f32) / area0
    w1 = (num_pos1 > 0.001).astype(f32) / area1

    ce0 = -w0 * (pos_sum0 - num_pos0 * lse0) / np.maximum(num_pos0, 1.0)
    ce1 = -w1 * (pos_sum1 - num_pos1 * lse1) / np.maximum(num_pos1, 1.0)
    return np.float32(ce0.mean() + ce1.mean())


def run_hw(inputs, trace=False):
    nc = _get_nc()
    in_maps, diag = host_prep(inputs["pred1"], inputs["pred2"],
                              inputs["target1"], inputs["target2"])
    last_err = None
    for attempt in range(3):
        try:
            res = run_bass_kernel_spmd(nc, in_maps,
                                       core_ids=list(range(NCORES)),
                                       trace=trace)
            break
        except Exception as e:  # transient NRT device errors recover on retry
            last_err = e
            import time
            time.sleep(20 * (attempt + 1))
    else:
        raise last_err
    loss = host_post(res.results, diag, inputs["pind1"], inputs["pind2"],
                     inputs["tind1"], inputs["tind2"])
    return loss, res


def kernel(**inputs):
    loss, _ = run_hw(inputs, trace=False)
    return loss

